# revision 27
# baseline (speedup 1.0000x reference)
"""Single-head attention (B=8, S=2048, IN=1024, QD=128, VD=1024) on 8 TRN2
NeuronCores, data-parallel over batch (one batch element per core).

Math per core (batch b):
    q = x Wq + bq ; k = x Wk + bk ; v = x Wv + bv
    out = tanh(softmax(q k^T) v)

Layout strategy (all matmuls contract over the partition dim):
  - host pre-transposes x[b] -> xT [IN, S] so projections need no on-chip
    transpose. qT [QD, S] = Wq^T xT, kT likewise, v [S, VD] = xT^T Wv.
  - scores are built TRANSPOSED: sT [t, s] = kT^T qT, so exp(sT) ("E^T")
    is directly the stationary operand of the AV matmul:
        o [s, VD] = (E^T)^T v   (accumulated over 16 t-tiles in PSUM)
    and softmax needs no max-subtraction (|scores| <= ~21, exp is finite
    in fp32) and no transposes.
  - row-denominators come from an extra N=1 matmul per (s,t) tile with an
    all-ones rhs; normalization folds into the final tanh activation as a
    per-partition scale: out = tanh(o_raw * recip(denom)).

Dtypes: q/k/scores matmuls run in float32r (fp32 layout, ~11-bit mantissa
rounding on HW, 1 cycle/row vs fp32's 4; bf16 anywhere in the q/k path
fails the 2e-2 absmax gate). E and the AV matmul run in bf16. The
v-projection runs fully in bf16 (Wv loaded as bf16 from HBM, x converted
on-device by the vector engine): bf16 moving rows are ~5% faster than
f32r (216 vs 227 ns per 512-row matmul) and measured absmax err stays at
6.8e-3. fp8+DoubleRow was evaluated and rejected: real-HW throughput is
~2x bf16 (not the cost model's 4x), and the absmax gate forces >=3-term
error-compensated decompositions which erase the gain.

Pipelining: (1) phases B1/B2 run as two kt-half passes (PSUM partials
drained to SBUF, second pass adds in place) so the v-projection's first
half overlaps the xt4-7/wv DMA stream; (2) phase C interleaves block n's
scores+exp (per odd t, after the t's AV matmuls so the kT weight load
hides behind full-size matmuls) into block n-1's first AV accumulation
loop; (3) startup: the first q-projection consumes per-kt weight tiles
and 512-column x chunks so the PE starts after ~0.3 MB of DMA instead of
2 MB, with a burst of dummy warm-up matmuls (on a memset tile) issued
first to bring the PE out of its low-power state during the DMA wait.
"""

import numpy as np
import ml_dtypes

import concourse.bacc as bacc
import concourse.mybir as mybir
import concourse.tile as tile
from concourse.bass_utils import run_bass_kernel_spmd

B, S, IN, QD, VD = 8, 2048, 1024, 128, 1024
N_CORES = 8
P = 128
KT = IN // P          # 8 contraction tiles for projections
TT = S // P           # 16 t-tiles
S_BLK = 512           # s-block width for scores/E^T staging
N_BLK = S // S_BLK    # 4 blocks
SS = S_BLK // P       # 4 s-subtiles per block

F32 = mybir.dt.float32
F32R = mybir.dt.float32r
BF16 = mybir.dt.bfloat16

_CACHE: dict = {}


def _build():
    if "nc" in _CACHE:
        return _CACHE["nc"]

    nc = bacc.Bacc("TRN2", target_bir_lowering=False, debug=False,
                   num_devices=N_CORES)

    xT_d = nc.dram_tensor("xT", [IN, S], F32, kind="ExternalInput").ap()
    wq_d = nc.dram_tensor("wq", [P, KT, QD], F32, kind="ExternalInput").ap()
    wk_d = nc.dram_tensor("wk", [P, KT, QD], F32, kind="ExternalInput").ap()
    wv_d = nc.dram_tensor("wv", [P, KT, VD], BF16, kind="ExternalInput").ap()
    bq_d = nc.dram_tensor("bq", [QD], F32, kind="ExternalInput").ap()
    bk_d = nc.dram_tensor("bk", [QD], F32, kind="ExternalInput").ap()
    bv_d = nc.dram_tensor("bv", [VD], F32, kind="ExternalInput").ap()
    out_d = nc.dram_tensor("out", [S, VD], F32, kind="ExternalOutput").ap()

    with tile.TileContext(nc) as tc:
        with (
            tc.tile_pool(name="consts", bufs=1) as consts,
            tc.tile_pool(name="x0c", bufs=4) as p_x0c,
            tc.tile_pool(name="xt", bufs=KT - 1) as p_xt,
            tc.tile_pool(name="xb", bufs=4) as p_xb,
            tc.tile_pool(name="wqk", bufs=2 * KT) as p_wqk,
            tc.tile_pool(name="wv", bufs=KT) as p_wv,
            tc.tile_pool(name="qk", bufs=1) as p_qk,
            tc.tile_pool(name="v", bufs=TT) as p_v,
            tc.tile_pool(name="et", bufs=2 * TT) as p_et,
            tc.tile_pool(name="o", bufs=2) as p_o,
            tc.tile_pool(name="recip", bufs=4) as p_recip,
            tc.tile_pool(name="ps", bufs=8, space="PSUM") as ps,
        ):
            # ---- PE warm-up: dummy matmuls on a memset tile so the PE
            # p-state ramps to full clock during the initial DMA wait ----
            junk = consts.tile([P, 512], BF16, tag="junk")
            junk_w = consts.tile([P, P], BF16, tag="junk_w")
            nc.vector.memset(junk[:], 0.25)
            nc.vector.memset(junk_w[:], 0.25)
            warm_ps = ps.tile([P, 512], F32, tag="ps", name="warmps")
            NWARM = 3
            for i in range(NWARM):
                nc.tensor.matmul(warm_ps[:], junk_w[:], junk[:],
                                 start=(i == 0), stop=(i == NWARM - 1))

            # ---- constant / weight loads ----
            # dma_start triggers cost ~0.65us each on the issuing engine's
            # queue, so they are spread across three otherwise-idle queues:
            # Vector carries the tiny bias loads (so the bv broadcast — which
            # gates the in-order PE — is ready ~immediately), Scalar carries
            # the q/k weights, Sync streams x and Wv.
            wq_sb = [None] * KT
            wk_sb = [None] * KT

            def load_wqk(which, kt, eng):
                d_, lst = (wq_d, wq_sb) if which == "q" else (wk_d, wk_sb)
                t_ = p_wqk.tile([P, QD], F32R, tag="wqk",
                                name=f"w{which}{kt}")
                eng.dma_start(out=t_[:], in_=d_[:, kt, :].bitcast(F32R))
                lst[kt] = t_

            def wq_at(kt):
                return wq_sb[kt][:]

            def wk_at(kt):
                return wk_sb[kt][:]

            ones_sb = consts.tile([P, 1], BF16, tag="ones")
            nc.vector.memset(ones_sb[:], 1.0)
            ones_row = consts.tile([1, P], BF16, tag="ones_row")
            nc.vector.memset(ones_row[:], 1.0)

            bq_sb = consts.tile([P, 1], F32, tag="bq")
            bk_sb = consts.tile([P, 1], F32, tag="bk")
            bv_row = consts.tile([1, VD], F32, tag="bv_row")
            bv_sb = consts.tile([P, VD], F32, tag="bv")
            bv_row_bf = consts.tile([1, VD], BF16, tag="bv_row_bf")
            wv_sb = [None] * KT

            # tiny loads first, on the (idle) Activation queue
            nc.scalar.dma_start(out=bv_row[:],
                                in_=bv_d.rearrange("(o v) -> o v", o=1))
            nc.scalar.dma_start(out=bq_sb[:],
                                in_=bq_d.rearrange("(p o) -> p o", o=1))
            nc.scalar.dma_start(out=bk_sb[:],
                                in_=bk_d.rearrange("(p o) -> p o", o=1))
            nc.vector.tensor_copy(bv_row_bf[:], bv_row[:])

            def load_wv(kt):
                t_ = p_wv.tile([P, VD], BF16, tag="wv", name=f"wvt{kt}")
                nc.sync.dma_start(out=t_[:], in_=wv_d[:, kt, :])
                wv_sb[kt] = t_

            # xt0 in four 512-column chunks; xt1-7 whole tiles
            x0c = []
            xt_sb = [None] * KT

            load_wqk("q", 0, nc.sync)
            for c in range(4):
                t_ = p_x0c.tile([P, 512], F32R, tag="x0c", name=f"x0c{c}")
                nc.sync.dma_start(
                    out=t_[:],
                    in_=xT_d[0:P, c * 512:(c + 1) * 512].bitcast(F32R))
                x0c.append(t_)
            load_wqk("k", 0, nc.scalar)
            for kt in range(1, 4):
                load_wqk("q", kt, nc.scalar)
                load_wqk("k", kt, nc.scalar)

            # broadcast bv across partitions via a K=1 outer product (the PE
            # is in-order, so this must be unblocked before the projections)
            for c in range(VD // 512):
                bv_ps = ps.tile([P, 512], F32, tag="ps", name=f"bvps{c}")
                nc.tensor.matmul(bv_ps[:], ones_row[:],
                                 bv_row_bf[:, c * 512:(c + 1) * 512],
                                 start=True, stop=True)
                nc.vector.tensor_copy(bv_sb[:, c * 512:(c + 1) * 512],
                                      bv_ps[:])

            def load_xt(kt, eng=None):
                t_ = p_xt.tile([P, S], F32R, tag="xt", name=f"xt{kt}")
                (eng or nc.sync).dma_start(
                    out=t_[:],
                    in_=xT_d[kt * P:(kt + 1) * P, :].bitcast(F32R))
                xt_sb[kt] = t_

            # xt1 on the sync ring right after the xt0 chunks; xt2/xt3 on
            # the scalar ring so both rings stream x in parallel during the
            # q/k projection prefix
            load_xt(1)
            load_xt(2, nc.scalar)
            load_xt(3, nc.scalar)
            for kt in range(4, KT):
                load_wqk("q", kt, nc.scalar)
                load_wqk("k", kt, nc.scalar)
            # wv0-3 before xt4-7: needed by the first v-projection half-pass
            for wkt in range(KT // 2):
                load_wv(wkt)
            for kt in range(4, KT):
                load_xt(kt)

            def xt_at(kt, sc):
                """[P, 512] chunk sc of x k-tile kt (rhs of q/k proj)."""
                if kt == 0:
                    return x0c[sc][:]
                return xt_sb[kt][:, sc * 512:(sc + 1) * 512]

            # bf16 copies of x (vproj stationary operand): converting both
            # vproj operands to bf16 makes its moving rows ~5% faster than
            # f32r and halves the Wv DMA. 4 ring buffers: kt0-3 used by
            # v_pass(0), then reused for kt4-7 in v_pass(1).
            xb_sb = [None] * KT

            def conv_xb(kt):
                t_ = p_xb.tile([P, S], BF16, tag="xb", name=f"xb{kt}")
                if kt == 0:
                    for c in range(4):
                        nc.vector.tensor_copy(t_[:, c * 512:(c + 1) * 512],
                                              x0c[c][:].bitcast(F32))
                else:
                    nc.vector.tensor_copy(t_[:], xt_sb[kt][:].bitcast(F32))
                xb_sb[kt] = t_

            def xl_at(kt, t):
                """[P, 128] s-column block t of x k-tile kt (lhsT of vproj)."""
                return xb_sb[kt][:, t * P:(t + 1) * P]

            for kt in range(4):
                conv_xb(kt)

            # ---- phases B1/B2 as two kt-half passes ----
            qT_sb = p_qk.tile([P, S], F32R, tag="qT")
            kT_sb = p_qk.tile([P, S], F32R, tag="kT")
            NSC = S // 512  # 4
            NVC = VD // 512
            KH = KT // 2

            def proj_pass(half):
                k0 = half * KH
                q_ps = [ps.tile([P, 512], F32, tag="ps",
                                name=f"qps{half}_{i}") for i in range(NSC)]
                k_ps = [ps.tile([P, 512], F32, tag="ps",
                                name=f"kps{half}_{i}") for i in range(NSC)]
                for kt in range(k0, k0 + KH):
                    # all q chunks then all k chunks: one weight load per
                    # group instead of one per matmul
                    for sc in range(NSC):
                        nc.tensor.matmul(q_ps[sc][:], wq_at(kt),
                                         xt_at(kt, sc),
                                         start=(kt == k0),
                                         stop=(kt == k0 + KH - 1))
                    for sc in range(NSC):
                        nc.tensor.matmul(k_ps[sc][:], wk_at(kt),
                                         xt_at(kt, sc),
                                         start=(kt == k0),
                                         stop=(kt == k0 + KH - 1))
                for sc in range(NSC):
                    sl = slice(sc * 512, (sc + 1) * 512)
                    if half == 0:
                        nc.vector.tensor_scalar_add(qT_sb[:, sl], q_ps[sc][:],
                                                    bq_sb[:])
                        nc.vector.tensor_scalar_add(kT_sb[:, sl], k_ps[sc][:],
                                                    bk_sb[:])
                    else:
                        nc.vector.tensor_add(qT_sb[:, sl], q_ps[sc][:],
                                             qT_sb[:, sl])
                        nc.vector.tensor_add(kT_sb[:, sl], k_ps[sc][:],
                                             kT_sb[:, sl])

            proj_pass(0)

            # ---- phase C helper (defined early: scores for block 0 are
            # interleaved into phase B2's tail) ----
            def emit_scores_t(sb, t):
                s0 = sb * S_BLK
                st_ps = ps.tile([P, S_BLK], F32, tag="ps", name=f"stps{sb}_{t}")
                nc.tensor.matmul(st_ps[:],
                                 kT_sb[:, t * P:(t + 1) * P],
                                 qT_sb[:, s0:s0 + S_BLK],
                                 start=True, stop=True)
                et = p_et.tile([P, S_BLK], BF16, tag="et", name=f"et{sb}_{t}")
                nc.scalar.activation(out=et[:], in_=st_ps[:],
                                     func=mybir.ActivationFunctionType.Exp)
                return et

            # ---- phase B2: v [S, VD] = xT^T Wv + bv, stored bf16 ----
            v_sb = [p_v.tile([P, VD], BF16, tag="v", name=f"v{t}")
                    for t in range(TT)]
            et0 = []

            def v_pass(half, interleave0):
                k0 = half * KH
                for t in range(TT):
                    vt = v_sb[t]
                    if interleave0 and t >= TT - 8:
                        et0.append(emit_scores_t(0, len(et0)))
                    v_ps = [ps.tile([P, 512], F32, tag="ps",
                                    name=f"vps{half}_{t}_{vc}")
                            for vc in range(NVC)]
                    for kt in range(k0, k0 + KH):
                        xl = xl_at(kt, t)
                        for vc in range(NVC):
                            nc.tensor.matmul(
                                v_ps[vc][:], xl,
                                wv_sb[kt][:, vc * 512:(vc + 1) * 512],
                                start=(kt == k0), stop=(kt == k0 + KH - 1))
                    if interleave0 and t >= TT - 8:
                        et0.append(emit_scores_t(0, len(et0)))
                    for vc in range(NVC):
                        sl = slice(vc * 512, (vc + 1) * 512)
                        if half == 0:
                            nc.vector.tensor_add(vt[:, sl], v_ps[vc][:],
                                                 bv_sb[:, sl])
                        else:
                            nc.vector.tensor_add(vt[:, sl], v_ps[vc][:],
                                                 vt[:, sl])

            v_pass(0, interleave0=False)
            for kt in range(KT // 2, KT):
                load_wv(kt)
            for kt in range(4, KT):
                conv_xb(kt)
            proj_pass(1)
            v_pass(1, interleave0=True)

            # ---- phase C: software-pipelined over s-blocks ----
            # Block n's scores^T + exp are interleaved per odd t AFTER that
            # t's AV matmuls, so the scores weight load (fp32 kT, ~190 ns)
            # hides behind two full-size AV matmuls instead of the 1-row
            # denominator matmul.
            def emit_av_ss(sb, ss, et_tiles, interleave_sb=None,
                           interleave_base=0, serialize_vc=False):
                o_ps = [ps.tile([P, 512], F32, tag="ps", name=f"ops{sb}_{ss}_{i}")
                        for i in range(VD // 512)]
                d_ps = ps.tile([P, 1], F32, tag="ps", name=f"dps{sb}_{ss}")
                nxt = []
                recip = p_recip.tile([P, 1], F32, tag="recip",
                                     name=f"recip{sb}_{ss}")
                o_sb = p_o.tile([P, VD], F32, tag="o", name=f"osb{sb}_{ss}")
                srow = sb * S_BLK + ss * P

                def drain_vc(vc, half=None):
                    if half is None:
                        cols = slice(vc * 512, (vc + 1) * 512)
                        pcols = slice(0, 512)
                    else:
                        cols = slice(vc * 512 + half * 256,
                                     vc * 512 + (half + 1) * 256)
                        pcols = slice(half * 256, (half + 1) * 256)
                    nc.scalar.activation(
                        out=o_sb[:, cols],
                        in_=o_ps[vc][:, pcols],
                        func=mybir.ActivationFunctionType.Tanh,
                        scale=recip[:])
                    nc.sync.dma_start(out=out_d[srow:srow + P, cols],
                                      in_=o_sb[:, cols])

                if not serialize_vc:
                    for t in range(TT):
                        # order vc0, d, vc1: the two cheap weight loads (E
                        # for d and vc1) amortize under vc0's 216 ns, and
                        # vc1 then covers the fat fp32 kT weight load of an
                        # interleaved scores matmul
                        lhs = et_tiles[t][:, ss * P:(ss + 1) * P]
                        nc.tensor.matmul(o_ps[0][:], lhs,
                                         v_sb[t][:, 0:512],
                                         start=(t == 0), stop=(t == TT - 1))
                        nc.tensor.matmul(d_ps[:], lhs, ones_sb[:],
                                         start=(t == 0), stop=(t == TT - 1))
                        nc.tensor.matmul(o_ps[1][:], lhs,
                                         v_sb[t][:, 512:1024],
                                         start=(t == 0), stop=(t == TT - 1))
                        if interleave_sb is not None and t % 2 == 1:
                            nxt.append(emit_scores_t(
                                interleave_sb, interleave_base + t // 2))
                    nc.vector.reciprocal(recip[:], d_ps[:])
                    for vc in range(VD // 512):
                        drain_vc(vc)
                else:
                    # tail variant: finish vc0 (and the denominator) first so
                    # its tanh+DMA overlap vc1's accumulation; vc1 drains in
                    # two 256-column halves to shorten the final unhidden
                    # tanh+DMA.
                    for t in range(TT):
                        lhs = et_tiles[t][:, ss * P:(ss + 1) * P]
                        nc.tensor.matmul(d_ps[:], lhs, ones_sb[:],
                                         start=(t == 0), stop=(t == TT - 1))
                        nc.tensor.matmul(o_ps[0][:], lhs, v_sb[t][:, 0:512],
                                         start=(t == 0), stop=(t == TT - 1))
                    nc.vector.reciprocal(recip[:], d_ps[:])
                    drain_vc(0)
                    # final 512 columns in two 256-wide PSUM banks: bank A's
                    # tanh+DMA overlap bank B's accumulation, shortening the
                    # unhidden tail to one 256-column drain
                    o1 = [ps.tile([P, 256], F32, tag="ps", name=f"otl{h}")
                          for h in range(2)]

                    def drain_h(h):
                        cols = slice(512 + h * 256, 512 + (h + 1) * 256)
                        nc.scalar.activation(
                            out=o_sb[:, cols], in_=o1[h][:],
                            func=mybir.ActivationFunctionType.Tanh,
                            scale=recip[:])
                        nc.sync.dma_start(out=out_d[srow:srow + P, cols],
                                          in_=o_sb[:, cols])

                    for h in range(2):
                        for t in range(TT):
                            lhs = et_tiles[t][:, ss * P:(ss + 1) * P]
                            nc.tensor.matmul(
                                o1[h][:], lhs,
                                v_sb[t][:, 512 + h * 256:512 + (h + 1) * 256],
                                start=(t == 0), stop=(t == TT - 1))
                        drain_h(h)
                return nxt

            et_cur = et0
            for sb in range(N_BLK):
                nxt_sb = sb + 1 if sb + 1 < N_BLK else None
                et_nxt = emit_av_ss(sb, 0, et_cur, interleave_sb=nxt_sb)
                for ssi in range(1, SS):
                    ilv = nxt_sb if ssi == 1 else None
                    et_nxt += emit_av_ss(sb, ssi, et_cur, interleave_sb=ilv,
                                         interleave_base=8,
                                         serialize_vc=(sb == N_BLK - 1
                                                       and ssi == SS - 1))
                et_cur = et_nxt

    nc.compile()
    _CACHE["nc"] = nc
    return nc


def _prep_inputs(x, Wq, bq, Wk, bk, Wv, bv):
    x = np.asarray(x, np.float32)
    xT = np.ascontiguousarray(x.transpose(0, 2, 1))          # [B, IN, S]
    wq = np.ascontiguousarray(
        np.asarray(Wq, np.float32).reshape(KT, P, QD).transpose(1, 0, 2))
    wk = np.ascontiguousarray(
        np.asarray(Wk, np.float32).reshape(KT, P, QD).transpose(1, 0, 2))
    wv = np.ascontiguousarray(
        np.asarray(Wv, np.float32).reshape(KT, P, VD).transpose(1, 0, 2)
    ).astype(ml_dtypes.bfloat16)
    shared = {
        "wq": wq, "wk": wk, "wv": wv,
        "bq": np.asarray(bq, np.float32),
        "bk": np.asarray(bk, np.float32),
        "bv": np.asarray(bv, np.float32),
    }
    return [dict(shared, xT=xT[c]) for c in range(N_CORES)]


def run(x, Wq, bq, Wk, bk, Wv, bv, trace=False):
    nc = _build()
    in_maps = _prep_inputs(x, Wq, bq, Wk, bk, Wv, bv)
    res = run_bass_kernel_spmd(nc, in_maps, list(range(N_CORES)), trace=trace)
    out = np.stack([res.results[c]["out"] for c in range(N_CORES)])
    return out.astype(np.float32), res


def kernel(x, Wq, bq, Wk, bk, Wv, bv):
    out, _ = run(x, Wq, bq, Wk, bk, Wv, bv, trace=False)
    return out


# revision 29
# speedup vs baseline: 1.0557x; 1.0557x over previous
"""Single-head attention (B=8, S=2048, IN=1024, QD=128, VD=1024) on 8 TRN2
NeuronCores, data-parallel over batch (one batch element per core).

Math per core (batch b):
    q = x Wq + bq ; k = x Wk + bk ; v = x Wv + bv
    out = tanh(softmax(q k^T) v)

Layout strategy (all matmuls contract over the partition dim):
  - host pre-transposes x[b] -> xT [IN, S] so projections need no on-chip
    transpose. qT [QD, S] = Wq^T xT, kT likewise, v [S, VD] = xT^T Wv.
  - scores are built TRANSPOSED: sT [t, s] = kT^T qT, so exp(sT) ("E^T")
    is directly the stationary operand of the AV matmul:
        o [s, VD] = (E^T)^T v   (accumulated over 16 t-tiles in PSUM)
    and softmax needs no max-subtraction (|scores| <= ~21, exp is finite
    in fp32) and no transposes.
  - row-denominators come from an extra N=1 matmul per (s,t) tile with an
    all-ones rhs; normalization folds into the final tanh activation as a
    per-partition scale: out = tanh(o_raw * recip(denom)).

Dtypes: q/k/scores matmuls run in float32r (fp32 layout, ~11-bit mantissa
rounding on HW, 1 cycle/row vs fp32's 4; bf16 anywhere in the q/k path
fails the 2e-2 absmax gate). E and the AV matmul run in bf16. The
v-projection runs fully in bf16 (Wv loaded as bf16 from HBM, x converted
on-device by the vector engine): bf16 moving rows are ~5% faster than
f32r (216 vs 227 ns per 512-row matmul) and measured absmax err stays at
6.8e-3. fp8+DoubleRow was evaluated and rejected: real-HW throughput is
~2x bf16 (not the cost model's 4x), and the absmax gate forces >=3-term
error-compensated decompositions which erase the gain.

Pipelining: (1) phases B1/B2 run as two kt-half passes (PSUM partials
drained to SBUF, second pass adds in place) so the v-projection's first
half overlaps the xt4-7/wv DMA stream; (2) phase C interleaves block n's
scores+exp (per odd t, after the t's AV matmuls so the kT weight load
hides behind full-size matmuls) into block n-1's first AV accumulation
loop; (3) startup: the first q-projection consumes per-kt weight tiles
and 512-column x chunks so the PE starts after ~0.3 MB of DMA instead of
2 MB, with a burst of dummy warm-up matmuls (on a memset tile) issued
first to bring the PE out of its low-power state during the DMA wait.
"""

import numpy as np
import ml_dtypes

import concourse.bacc as bacc
import concourse.mybir as mybir
import concourse.tile as tile
from concourse.bass_utils import run_bass_kernel_spmd

B, S, IN, QD, VD = 8, 2048, 1024, 128, 1024
N_CORES = 8
P = 128
KT = IN // P          # 8 contraction tiles for projections
TT = S // P           # 16 t-tiles
S_BLK = 512           # s-block width for scores/E^T staging
N_BLK = S // S_BLK    # 4 blocks
SS = S_BLK // P       # 4 s-subtiles per block

F32 = mybir.dt.float32
F32R = mybir.dt.float32r
BF16 = mybir.dt.bfloat16

_CACHE: dict = {}


def _build():
    if "nc" in _CACHE:
        return _CACHE["nc"]

    nc = bacc.Bacc("TRN2", target_bir_lowering=False, debug=False,
                   num_devices=N_CORES)

    xT_d = nc.dram_tensor("xT", [IN, S], F32, kind="ExternalInput").ap()
    wq_d = nc.dram_tensor("wq", [P, KT, QD], F32, kind="ExternalInput").ap()
    wk_d = nc.dram_tensor("wk", [P, KT, QD], F32, kind="ExternalInput").ap()
    wv_d = nc.dram_tensor("wv", [P, KT, VD], BF16, kind="ExternalInput").ap()
    bq_d = nc.dram_tensor("bq", [QD], F32, kind="ExternalInput").ap()
    bk_d = nc.dram_tensor("bk", [QD], F32, kind="ExternalInput").ap()
    bv_d = nc.dram_tensor("bv", [VD], F32, kind="ExternalInput").ap()
    out_d = nc.dram_tensor("out", [S, VD], F32, kind="ExternalOutput").ap()

    with tile.TileContext(nc) as tc:
        with (
            tc.tile_pool(name="consts", bufs=1) as consts,
            tc.tile_pool(name="x0c", bufs=4) as p_x0c,
            tc.tile_pool(name="xt", bufs=KT - 1) as p_xt,
            tc.tile_pool(name="xb", bufs=4) as p_xb,
            tc.tile_pool(name="wqk", bufs=2 * KT) as p_wqk,
            tc.tile_pool(name="wv", bufs=KT) as p_wv,
            tc.tile_pool(name="qk", bufs=1) as p_qk,
            tc.tile_pool(name="v", bufs=TT) as p_v,
            tc.tile_pool(name="et", bufs=2 * TT) as p_et,
            tc.tile_pool(name="o", bufs=2) as p_o,
            tc.tile_pool(name="recip", bufs=4) as p_recip,
            tc.tile_pool(name="ps", bufs=8, space="PSUM") as ps,
        ):
            # ---- PE warm-up: dummy matmuls on a memset tile so the PE
            # p-state ramps to full clock during the initial DMA wait ----
            junk = consts.tile([P, 512], BF16, tag="junk")
            junk_w = consts.tile([P, P], BF16, tag="junk_w")
            nc.vector.memset(junk[:], 0.25)
            nc.vector.memset(junk_w[:], 0.25)
            warm_ps = ps.tile([P, 512], F32, tag="ps", name="warmps")
            NWARM = 3
            for i in range(NWARM):
                nc.tensor.matmul(warm_ps[:], junk_w[:], junk[:],
                                 start=(i == 0), stop=(i == NWARM - 1))

            # ---- constant / weight loads ----
            # dma_start triggers cost ~0.65us each on the issuing engine's
            # queue, so they are spread across three otherwise-idle queues:
            # Vector carries the tiny bias loads (so the bv broadcast — which
            # gates the in-order PE — is ready ~immediately), Scalar carries
            # the q/k weights, Sync streams x and Wv.
            wq_sb = [None] * KT
            wk_sb = [None] * KT

            def load_wqk(which, kt, eng):
                d_, lst = (wq_d, wq_sb) if which == "q" else (wk_d, wk_sb)
                t_ = p_wqk.tile([P, QD], F32R, tag="wqk",
                                name=f"w{which}{kt}")
                eng.dma_start(out=t_[:], in_=d_[:, kt, :].bitcast(F32R))
                lst[kt] = t_

            def wq_at(kt):
                return wq_sb[kt][:]

            def wk_at(kt):
                return wk_sb[kt][:]

            ones_sb = consts.tile([P, 1], BF16, tag="ones")
            nc.vector.memset(ones_sb[:], 1.0)
            ones_row = consts.tile([1, P], BF16, tag="ones_row")
            nc.vector.memset(ones_row[:], 1.0)

            bq_sb = consts.tile([P, 1], F32, tag="bq")
            bk_sb = consts.tile([P, 1], F32, tag="bk")
            bv_row = consts.tile([1, VD], F32, tag="bv_row")
            bv_sb = consts.tile([P, VD], F32, tag="bv")
            bv_row_bf = consts.tile([1, VD], BF16, tag="bv_row_bf")
            wv_sb = [None] * KT

            # tiny loads first, on the (idle) Activation queue
            nc.scalar.dma_start(out=bv_row[:],
                                in_=bv_d.rearrange("(o v) -> o v", o=1))
            nc.scalar.dma_start(out=bq_sb[:],
                                in_=bq_d.rearrange("(p o) -> p o", o=1))
            nc.scalar.dma_start(out=bk_sb[:],
                                in_=bk_d.rearrange("(p o) -> p o", o=1))
            nc.vector.tensor_copy(bv_row_bf[:], bv_row[:])

            def load_wv(kt):
                t_ = p_wv.tile([P, VD], BF16, tag="wv", name=f"wvt{kt}")
                nc.sync.dma_start(out=t_[:], in_=wv_d[:, kt, :])
                wv_sb[kt] = t_

            # xt0 in four 512-column chunks; xt1-7 whole tiles
            x0c = []
            xt_sb = [None] * KT

            load_wqk("q", 0, nc.sync)
            for c in range(4):
                t_ = p_x0c.tile([P, 512], F32R, tag="x0c", name=f"x0c{c}")
                nc.sync.dma_start(
                    out=t_[:],
                    in_=xT_d[0:P, c * 512:(c + 1) * 512].bitcast(F32R))
                x0c.append(t_)
            load_wqk("k", 0, nc.scalar)
            for kt in range(1, 4):
                load_wqk("q", kt, nc.scalar)
                load_wqk("k", kt, nc.scalar)

            # broadcast bv across partitions via a K=1 outer product (the PE
            # is in-order, so this must be unblocked before the projections)
            for c in range(VD // 512):
                bv_ps = ps.tile([P, 512], F32, tag="ps", name=f"bvps{c}")
                nc.tensor.matmul(bv_ps[:], ones_row[:],
                                 bv_row_bf[:, c * 512:(c + 1) * 512],
                                 start=True, stop=True)
                nc.vector.tensor_copy(bv_sb[:, c * 512:(c + 1) * 512],
                                      bv_ps[:])

            def load_xt(kt, eng=None):
                t_ = p_xt.tile([P, S], F32R, tag="xt", name=f"xt{kt}")
                (eng or nc.sync).dma_start(
                    out=t_[:],
                    in_=xT_d[kt * P:(kt + 1) * P, :].bitcast(F32R))
                xt_sb[kt] = t_

            # xt1 on the sync ring right after the xt0 chunks; xt2/xt3 on
            # the scalar ring so both rings stream x in parallel during the
            # q/k projection prefix
            load_xt(1)
            load_xt(2, nc.scalar)
            load_xt(3, nc.scalar)
            for kt in range(4, KT):
                load_wqk("q", kt, nc.scalar)
                load_wqk("k", kt, nc.scalar)
            # wv0-3 before xt4-7: needed by the first v-projection half-pass
            for wkt in range(KT // 2):
                load_wv(wkt)
            for kt in range(4, KT):
                load_xt(kt)

            def xt_at(kt, sc):
                """[P, 512] chunk sc of x k-tile kt (rhs of q/k proj)."""
                if kt == 0:
                    return x0c[sc][:]
                return xt_sb[kt][:, sc * 512:(sc + 1) * 512]

            # bf16 copies of x (vproj stationary operand): converting both
            # vproj operands to bf16 makes its moving rows ~5% faster than
            # f32r and halves the Wv DMA. 4 ring buffers: kt0-3 used by
            # v_pass(0), then reused for kt4-7 in v_pass(1).
            xb_sb = [None] * KT

            def conv_xb(kt):
                t_ = p_xb.tile([P, S], BF16, tag="xb", name=f"xb{kt}")
                if kt == 0:
                    for c in range(4):
                        nc.vector.tensor_copy(t_[:, c * 512:(c + 1) * 512],
                                              x0c[c][:].bitcast(F32))
                else:
                    nc.vector.tensor_copy(t_[:], xt_sb[kt][:].bitcast(F32))
                xb_sb[kt] = t_

            def xl_at(kt, t):
                """[P, 128] s-column block t of x k-tile kt (lhsT of vproj)."""
                return xb_sb[kt][:, t * P:(t + 1) * P]

            for kt in range(4):
                conv_xb(kt)

            # ---- phases B1/B2 as two kt-half passes ----
            qT_sb = p_qk.tile([P, S], F32R, tag="qT")
            kT_sb = p_qk.tile([P, S], F32R, tag="kT")
            NSC = S // 512  # 4
            NVC = VD // 512
            KH = KT // 2

            def proj_pass(half):
                k0 = half * KH
                q_ps = [ps.tile([P, 512], F32, tag="ps",
                                name=f"qps{half}_{i}") for i in range(NSC)]
                k_ps = [ps.tile([P, 512], F32, tag="ps",
                                name=f"kps{half}_{i}") for i in range(NSC)]
                for kt in range(k0, k0 + KH):
                    # all q chunks then all k chunks: one weight load per
                    # group instead of one per matmul
                    for sc in range(NSC):
                        nc.tensor.matmul(q_ps[sc][:], wq_at(kt),
                                         xt_at(kt, sc),
                                         start=(kt == k0),
                                         stop=(kt == k0 + KH - 1))
                    for sc in range(NSC):
                        nc.tensor.matmul(k_ps[sc][:], wk_at(kt),
                                         xt_at(kt, sc),
                                         start=(kt == k0),
                                         stop=(kt == k0 + KH - 1))
                for sc in range(NSC):
                    sl = slice(sc * 512, (sc + 1) * 512)
                    if half == 0:
                        nc.vector.tensor_scalar_add(qT_sb[:, sl], q_ps[sc][:],
                                                    bq_sb[:])
                        nc.vector.tensor_scalar_add(kT_sb[:, sl], k_ps[sc][:],
                                                    bk_sb[:])
                    else:
                        nc.vector.tensor_add(qT_sb[:, sl], q_ps[sc][:],
                                             qT_sb[:, sl])
                        nc.vector.tensor_add(kT_sb[:, sl], k_ps[sc][:],
                                             kT_sb[:, sl])

            proj_pass(0)

            # ---- phase C helper (defined early: scores for block 0 are
            # interleaved into phase B2's tail) ----
            def emit_scores_t(sb, t):
                s0 = sb * S_BLK
                st_ps = ps.tile([P, S_BLK], F32, tag="ps", name=f"stps{sb}_{t}")
                nc.tensor.matmul(st_ps[:],
                                 kT_sb[:, t * P:(t + 1) * P],
                                 qT_sb[:, s0:s0 + S_BLK],
                                 start=True, stop=True)
                et = p_et.tile([P, S_BLK], BF16, tag="et", name=f"et{sb}_{t}")
                nc.scalar.activation(out=et[:], in_=st_ps[:],
                                     func=mybir.ActivationFunctionType.Exp)
                return et

            # ---- phase B2: v [S, VD] = xT^T Wv + bv, stored bf16 ----
            # column VD holds a constant 1.0: the AV matmul then produces the
            # softmax denominator as a free extra output column, removing the
            # 256 one-row denominator matmuls (each of which cost ~21 ns of
            # PE pipeline refill on the following matmul)
            v_sb = [p_v.tile([P, VD + 1], BF16, tag="v", name=f"v{t}")
                    for t in range(TT)]
            for t in range(TT):
                nc.vector.memset(v_sb[t][:, VD:VD + 1], 1.0)
            et0 = []

            def v_pass(half, interleave0):
                k0 = half * KH
                for t in range(TT):
                    vt = v_sb[t]
                    if interleave0 and t >= TT - 8:
                        et0.append(emit_scores_t(0, len(et0)))
                    v_ps = [ps.tile([P, 512], F32, tag="ps",
                                    name=f"vps{half}_{t}_{vc}")
                            for vc in range(NVC)]
                    for kt in range(k0, k0 + KH):
                        xl = xl_at(kt, t)
                        for vc in range(NVC):
                            nc.tensor.matmul(
                                v_ps[vc][:], xl,
                                wv_sb[kt][:, vc * 512:(vc + 1) * 512],
                                start=(kt == k0), stop=(kt == k0 + KH - 1))
                    if interleave0 and t >= TT - 8:
                        et0.append(emit_scores_t(0, len(et0)))
                    for vc in range(NVC):
                        sl = slice(vc * 512, (vc + 1) * 512)
                        if half == 0:
                            nc.vector.tensor_add(vt[:, sl], v_ps[vc][:],
                                                 bv_sb[:, sl])
                        else:
                            nc.vector.tensor_add(vt[:, sl], v_ps[vc][:],
                                                 vt[:, sl])

            v_pass(0, interleave0=False)
            for kt in range(KT // 2, KT):
                load_wv(kt)
            for kt in range(4, KT):
                conv_xb(kt)
            proj_pass(1)
            v_pass(1, interleave0=True)

            # ---- phase C: software-pipelined over s-blocks ----
            # Block n's scores^T + exp are interleaved per odd t AFTER that
            # t's AV matmuls, so the scores weight load (fp32 kT, ~190 ns)
            # hides behind two full-size AV matmuls instead of the 1-row
            # denominator matmul.
            # chunk layout over the VD+1 v columns (ones col last); chunk 2's
            # final PSUM column is the denominator
            CW = [(0, 342), (342, 684), (684, VD + 1)]

            def emit_av_ss(sb, ss, et_tiles, interleave_sb=None,
                           interleave_base=0, serialize_vc=False):
                o_ps = [ps.tile([P, c1 - c0], F32, tag="ps",
                                name=f"ops{sb}_{ss}_{i}")
                        for i, (c0, c1) in enumerate(CW)]
                nxt = []
                recip = p_recip.tile([P, 1], F32, tag="recip",
                                     name=f"recip{sb}_{ss}")
                o_sb = p_o.tile([P, VD], F32, tag="o", name=f"osb{sb}_{ss}")
                srow = sb * S_BLK + ss * P
                DCOL = VD - CW[2][0]  # denominator's local column in chunk 2

                def mm_chunk(ci, t, lhs):
                    c0, c1 = CW[ci]
                    nc.tensor.matmul(o_ps[ci][:], lhs, v_sb[t][:, c0:c1],
                                     start=(t == 0), stop=(t == TT - 1))

                def drain_chunk(ci):
                    c0, c1 = CW[ci]
                    w = min(c1, VD) - c0  # drop the ones column in chunk 2
                    nc.scalar.activation(
                        out=o_sb[:, c0:c0 + w],
                        in_=o_ps[ci][:, 0:w],
                        func=mybir.ActivationFunctionType.Tanh,
                        scale=recip[:])
                    nc.sync.dma_start(out=out_d[srow:srow + P, c0:c0 + w],
                                      in_=o_sb[:, c0:c0 + w])

                if not serialize_vc:
                    for t in range(TT):
                        lhs = et_tiles[t][:, ss * P:(ss + 1) * P]
                        for ci in range(3):
                            mm_chunk(ci, t, lhs)
                        if interleave_sb is not None and t % 2 == 1:
                            nxt.append(emit_scores_t(
                                interleave_sb, interleave_base + t // 2))
                    nc.vector.reciprocal(recip[:],
                                         o_ps[2][:, DCOL:DCOL + 1])
                    for ci in range(3):
                        drain_chunk(ci)
                else:
                    # tail variant: chunk 2 (which carries the denominator)
                    # accumulates first so recip + its drain overlap chunk
                    # 0/1 accumulation; the final unhidden drain is one
                    # ~342-column tanh+DMA
                    for t in range(TT):
                        lhs = et_tiles[t][:, ss * P:(ss + 1) * P]
                        mm_chunk(2, t, lhs)
                    nc.vector.reciprocal(recip[:],
                                         o_ps[2][:, DCOL:DCOL + 1])
                    drain_chunk(2)
                    for t in range(TT):
                        lhs = et_tiles[t][:, ss * P:(ss + 1) * P]
                        mm_chunk(0, t, lhs)
                    drain_chunk(0)
                    for t in range(TT):
                        lhs = et_tiles[t][:, ss * P:(ss + 1) * P]
                        mm_chunk(1, t, lhs)
                    drain_chunk(1)
                return nxt

            et_cur = et0
            for sb in range(N_BLK):
                nxt_sb = sb + 1 if sb + 1 < N_BLK else None
                et_nxt = emit_av_ss(sb, 0, et_cur, interleave_sb=nxt_sb)
                for ssi in range(1, SS):
                    ilv = nxt_sb if ssi == 1 else None
                    et_nxt += emit_av_ss(sb, ssi, et_cur, interleave_sb=ilv,
                                         interleave_base=8,
                                         serialize_vc=(sb == N_BLK - 1
                                                       and ssi == SS - 1))
                et_cur = et_nxt

    nc.compile()
    _CACHE["nc"] = nc
    return nc


def _prep_inputs(x, Wq, bq, Wk, bk, Wv, bv):
    x = np.asarray(x, np.float32)
    xT = np.ascontiguousarray(x.transpose(0, 2, 1))          # [B, IN, S]
    wq = np.ascontiguousarray(
        np.asarray(Wq, np.float32).reshape(KT, P, QD).transpose(1, 0, 2))
    wk = np.ascontiguousarray(
        np.asarray(Wk, np.float32).reshape(KT, P, QD).transpose(1, 0, 2))
    wv = np.ascontiguousarray(
        np.asarray(Wv, np.float32).reshape(KT, P, VD).transpose(1, 0, 2)
    ).astype(ml_dtypes.bfloat16)
    shared = {
        "wq": wq, "wk": wk, "wv": wv,
        "bq": np.asarray(bq, np.float32),
        "bk": np.asarray(bk, np.float32),
        "bv": np.asarray(bv, np.float32),
    }
    return [dict(shared, xT=xT[c]) for c in range(N_CORES)]


def run(x, Wq, bq, Wk, bk, Wv, bv, trace=False):
    nc = _build()
    in_maps = _prep_inputs(x, Wq, bq, Wk, bk, Wv, bv)
    res = run_bass_kernel_spmd(nc, in_maps, list(range(N_CORES)), trace=trace)
    out = np.stack([res.results[c]["out"] for c in range(N_CORES)])
    return out.astype(np.float32), res


def kernel(x, Wq, bq, Wk, bk, Wv, bv):
    out, _ = run(x, Wq, bq, Wk, bk, Wv, bv, trace=False)
    return out
